# revision 25
# baseline (speedup 1.0000x reference)
"""BatchMatchedMSELoss on 8 Trainium2 NeuronCores.

loss = mean(concat(row_min, col_min)) of the (B,B) pairwise-MSE matrix
  mse[i,j] = (||x_i||^2 + ||y_j||^2 - 2 x_i.y_j) / D,  B=8192, D=1024.

Sharding: input rows split across 8 cores (1024 rows each); every core
computes its (1024, 8192) tile of D*mse = sqx[i] + sqy[j] - 2*cross via
bf16 matmuls with fp32 PSUM accumulation. The host hands each core
contraction-major bf16 operands (pure layout/dtype prep — the TensorE
stream is then pure matmul) and the sq terms ride the contraction as a
K=4 tail tile of bf16 hi/lo rows. Row mins leave the device complete;
column partial mins (truncated to 32 partitions on device) are combined
on the host along with the final mean.
"""

import numpy as np
import ml_dtypes

import concourse.bass as bass
import concourse.tile as tile
import concourse.mybir as mybir
from concourse.bass import ts
from concourse.bass_utils import run_bass_kernel_spmd

FP32 = mybir.dt.float32
BF16 = mybir.dt.bfloat16
AL = mybir.AluOpType
AX = mybir.AxisListType

B = 8192          # batch (rows of input and target)
D = 1024          # feature dim (contraction)
NCORES = 8
RPC = B // NCORES  # rows per core = 1024
P = 128
MT = RPC // P      # 8 row tiles per core
DT = D // P        # 8 contraction tiles
CHUNK = 1024       # column chunk
NCH = B // CHUNK   # 8 chunks
HALF = 512         # max moving free dim per matmul / one PSUM bank


def _legalize_waits(nc, max_waits=1):
    """walrus codegen in this container rejects instructions carrying more
    than one sync-wait command. Split extra waits onto standalone
    EventSemaphore instructions (same engine, immediately before), which is
    exactly what engine.wait_ge() emits."""
    n = 0
    for f in nc.m.functions:
        for bb in f.blocks:
            insts = bb.instructions
            out = []
            for inst in insts:
                si = inst.sync_info
                if si is not None and si.on_wait and len(si.on_wait) > max_waits:
                    waits = list(si.on_wait)
                    extra, keep = waits[:-max_waits], waits[-max_waits:]
                    for w in extra:
                        n += 1
                        ev = mybir.InstEventSemaphore(
                            name=f"legwait-{n}-{inst.name}", ins=[], outs=[]
                        )
                        ev.engine = inst.engine
                        ev.sync_info = mybir.SyncInfo(on_wait=[w], on_update=[])
                        out.append(ev)
                    inst.sync_info = mybir.SyncInfo(
                        on_wait=keep, on_update=list(si.on_update)
                    )
                out.append(inst)
            bb.instructions = out
    return n


def build_bass(legalize: bool = True) -> bass.Bass:
    nc = bass.Bass()
    # xt = bf16((-2 * X_shard).T) [D, RPC]; yt = bf16(Y.T) [D, B]
    xt = nc.dram_tensor("xt", [D, RPC], BF16, kind="ExternalInput")
    yt = nc.dram_tensor("yt", [D, B], BF16, kind="ExternalInput")
    # K=4 tail: thinx rows = [sqx_hi; sqx_lo; 1; 1], thiny = [1; 1; sqy_hi; sqy_lo]
    thinx = nc.dram_tensor("thinx", [4, RPC], BF16, kind="ExternalInput")
    thiny = nc.dram_tensor("thiny", [4, B], BF16, kind="ExternalInput")
    rowmin_d = nc.dram_tensor("rowmin", [P, MT * NCH * 2], FP32, kind="ExternalOutput")
    # partition-min truncated at 32 rows on device; host finishes the min
    colmin_d = nc.dram_tensor("colmin", [32, B], FP32, kind="ExternalOutput")

    with tile.TileContext(nc) as tc:
        with (
            tc.tile_pool(name="consts", bufs=1) as consts,
            tc.tile_pool(name="yt8", bufs=2) as ytp,
            tc.tile_pool(name="thinp", bufs=2) as thinp,
            tc.tile_pool(name="work", bufs=3) as work,
            tc.tile_pool(name="pmm", bufs=2, space=bass.MemorySpace.PSUM) as pmm,
        ):
            rowmin_ch = consts.tile([P, MT * NCH * 2], FP32)
            thinX = consts.tile([4, RPC], BF16)
            nc.sync.dma_start(out=thinX[:], in_=thinx[:, :])
            XT = [
                consts.tile([P, RPC], BF16, tag=f"xt{d}", name=f"xt{d}")
                for d in range(DT)
            ]

            # warm the PE clock (HAM) with throwaway matmuls while loads land
            warm = consts.tile([P, P + HALF], BF16)
            nc.vector.memset(warm[:], 0.0)
            wps = pmm.tile([P, 2 * CHUNK], FP32, tag="ps")
            for w in range(44):
                nc.tensor.matmul(
                    wps[:, ts(w % 4, HALF)], warm[:, 0:P], warm[:, P : P + HALF],
                    start=True, stop=True,
                )

            # ---- Phase A: load X^T (already bf16) ----
            for dt in range(DT):
                for hf in range(4):  # split across DMA queues
                    nc.sync.dma_start(
                        out=XT[dt][:, ts(hf, RPC // 4)],
                        in_=xt[ts(dt, P), ts(hf, RPC // 4)],
                    )

            # ---- Phase B: stream column chunks of Y^T ----
            for ch in range(NCH):
                j0 = ch * CHUNK
                thinY = thinp.tile([4, CHUNK], BF16, tag="thiny")
                nc.sync.dma_start(out=thinY[:], in_=thiny[:, j0 : j0 + CHUNK])
                yts = []
                nq = 4 if ch == 0 else 2
                for dt in range(DT):
                    ytile = ytp.tile([P, CHUNK], BF16, tag=f"yt{dt}", name=f"yt{dt}")
                    for hf in range(nq):
                        w = CHUNK // nq
                        nc.sync.dma_start(
                            out=ytile[:, ts(hf, w)],
                            in_=yt[ts(dt, P), j0 + hf * w : j0 + (hf + 1) * w],
                        )
                    yts.append(ytile)

                colmin = work.tile([P, CHUNK], FP32, tag="colmin")
                for m in range(0, MT, 2):
                    # one 4-bank PSUM tile = 4 half-groups (2 mtiles x 2 halves)
                    ps = pmm.tile([P, 2 * CHUNK], FP32, tag="ps")
                    for g in range(4):
                        mm, h = m + g // 2, g % 2
                        hs = slice(h * HALF, (h + 1) * HALF)
                        gs = slice(g * HALF, (g + 1) * HALF)
                        for dt in range(DT):
                            nc.tensor.matmul(
                                ps[:, gs],
                                XT[dt][:, ts(mm, P)],
                                yts[dt][:, hs],
                                start=(dt == 0),
                                stop=False,
                            )
                        nc.tensor.matmul(
                            ps[:, gs], thinX[:, ts(mm, P)], thinY[:, hs],
                            start=False, stop=True,
                        )
                    for g in range(4):
                        mm, h = m + g // 2, g % 2
                        hs = slice(h * HALF, (h + 1) * HALF)
                        gs = slice(g * HALF, (g + 1) * HALF)
                        k = (mm * NCH + ch) * 2 + h
                        nc.vector.tensor_reduce(
                            out=rowmin_ch[:, k : k + 1], in_=ps[:, gs],
                            axis=AX.X, op=AL.min,
                        )
                        if mm == 0:
                            nc.vector.tensor_copy(colmin[:, hs], ps[:, gs])
                        else:
                            nc.vector.scalar_tensor_tensor(
                                colmin[:, hs], ps[:, gs], 0.0, colmin[:, hs],
                                op0=AL.bypass, op1=AL.min,
                            )

                # partial min across partitions (128 -> 32); host finishes
                for s in (64, 32):
                    tmp = work.tile([64, CHUNK], FP32, tag="tree")
                    nc.sync.dma_start(out=tmp[:s, :], in_=colmin[s : 2 * s, :])
                    nc.vector.tensor_tensor(
                        colmin[0:s, :], colmin[0:s, :], tmp[:s, :], AL.min
                    )
                nc.sync.dma_start(
                    out=colmin_d[:, j0 : j0 + CHUNK], in_=colmin[0:32, :]
                )

            nc.sync.dma_start(out=rowmin_d[:, :], in_=rowmin_ch[:, :])
    if legalize:
        _legalize_waits(nc)
    return nc


_NC_CACHE = None


def _get_nc():
    global _NC_CACHE
    if _NC_CACHE is None:
        _NC_CACHE = build_bass()
    return _NC_CACHE


def _hi_lo(v):
    hi = v.astype(ml_dtypes.bfloat16)
    lo = (v - hi.astype(np.float64)).astype(ml_dtypes.bfloat16)
    return hi, lo


def _prep_inputs(X, Y):
    """Host-side sharding/layout: contraction-major bf16 operands + packed
    sq rows."""
    yt = np.ascontiguousarray(Y.T.astype(ml_dtypes.bfloat16))
    sqy = (Y.astype(np.float64) ** 2).sum(axis=1)
    sqy_hi, sqy_lo = _hi_lo(sqy)
    ones_b = np.ones(B, dtype=ml_dtypes.bfloat16)
    thiny = np.ascontiguousarray(np.stack([ones_b, ones_b, sqy_hi, sqy_lo]))

    in_maps = []
    for c in range(NCORES):
        Xs = X[c * RPC : (c + 1) * RPC]
        xt = np.ascontiguousarray((-2.0 * Xs).T.astype(ml_dtypes.bfloat16))
        sqx = (Xs.astype(np.float64) ** 2).sum(axis=1)
        sqx_hi, sqx_lo = _hi_lo(sqx)
        ones_r = np.ones(RPC, dtype=ml_dtypes.bfloat16)
        thinx = np.ascontiguousarray(np.stack([sqx_hi, sqx_lo, ones_r, ones_r]))
        in_maps.append({"xt": xt, "yt": yt, "thinx": thinx, "thiny": thiny})
    return in_maps


def kernel(input, target):
    X = np.ascontiguousarray(np.asarray(input, dtype=np.float32))
    Y = np.ascontiguousarray(np.asarray(target, dtype=np.float32))
    assert X.shape == (B, D) and Y.shape == (B, D)

    nc = _get_nc()
    in_maps = _prep_inputs(X, Y)
    res = run_bass_kernel_spmd(nc, in_maps, core_ids=list(range(NCORES))).results

    row_sum = np.float64(0.0)
    col_parts = []
    for r in res:
        rm = r["rowmin"].reshape(P, MT, NCH * 2).min(axis=2)
        row_sum += rm.astype(np.float64).sum()
        col_parts.append(r["colmin"].min(axis=0))
    col_min = np.min(np.stack(col_parts), axis=0).astype(np.float64)
    loss = (row_sum + col_min.sum()) / D / (2 * B)
    return np.asarray(loss, dtype=np.float32)


# revision 28
# speedup vs baseline: 1.0916x; 1.0916x over previous
"""BatchMatchedMSELoss on 8 Trainium2 NeuronCores.

loss = mean(concat(row_min, col_min)) of the (B,B) pairwise-MSE matrix
  mse[i,j] = (||x_i||^2 + ||y_j||^2 - 2 x_i.y_j) / D,  B=8192, D=1024.

Sharding: input rows split across 8 cores (1024 rows each); every core
computes its (1024, 8192) tile of D*mse = sqx[i] + sqy[j] - 2*cross via
bf16 matmuls with fp32 PSUM accumulation. The host hands each core
contraction-major bf16 operands (pure layout/dtype prep — the TensorE
stream is then pure matmul) and the sq terms ride the contraction as a
K=4 tail tile of bf16 hi/lo rows. Row mins leave the device complete;
column partial mins (truncated to 32 partitions on device) are combined
on the host along with the final mean.
"""

import numpy as np
import ml_dtypes

import concourse.bass as bass
import concourse.tile as tile
import concourse.mybir as mybir
from concourse.bass import ts
from concourse.bass_utils import run_bass_kernel_spmd

FP32 = mybir.dt.float32
BF16 = mybir.dt.bfloat16
AL = mybir.AluOpType
AX = mybir.AxisListType

B = 8192          # batch (rows of input and target)
D = 1024          # feature dim (contraction)
NCORES = 8
RPC = B // NCORES  # rows per core = 1024
P = 128
MT = RPC // P      # 8 row tiles per core
DT = D // P        # 8 contraction tiles
CHUNK = 1024       # column chunk
NCH = B // CHUNK   # 8 chunks
HALF = 512         # max moving free dim per matmul / one PSUM bank


def _legalize_waits(nc, max_waits=1):
    """walrus codegen in this container rejects instructions carrying more
    than one sync-wait command. Split extra waits onto standalone
    EventSemaphore instructions (same engine, immediately before), which is
    exactly what engine.wait_ge() emits."""
    n = 0
    for f in nc.m.functions:
        for bb in f.blocks:
            insts = bb.instructions
            out = []
            for inst in insts:
                si = inst.sync_info
                if si is not None and si.on_wait and len(si.on_wait) > max_waits:
                    waits = list(si.on_wait)
                    extra, keep = waits[:-max_waits], waits[-max_waits:]
                    for w in extra:
                        n += 1
                        ev = mybir.InstEventSemaphore(
                            name=f"legwait-{n}-{inst.name}", ins=[], outs=[]
                        )
                        ev.engine = inst.engine
                        ev.sync_info = mybir.SyncInfo(on_wait=[w], on_update=[])
                        out.append(ev)
                    inst.sync_info = mybir.SyncInfo(
                        on_wait=keep, on_update=list(si.on_update)
                    )
                out.append(inst)
            bb.instructions = out
    return n


def build_bass(legalize: bool = True) -> bass.Bass:
    nc = bass.Bass()
    # xt = bf16((-2 * X_shard).T) [D, RPC]; yt = bf16(Y.T) [D, B]
    xt = nc.dram_tensor("xt", [D, RPC], BF16, kind="ExternalInput")
    yt = nc.dram_tensor("yt", [D, B], BF16, kind="ExternalInput")
    # K=4 tail: thinx rows = [sqx_hi; sqx_lo; 1; 1], thiny = [1; 1; sqy_hi; sqy_lo]
    thinx = nc.dram_tensor("thinx", [4, RPC], BF16, kind="ExternalInput")
    thiny = nc.dram_tensor("thiny", [4, B], BF16, kind="ExternalInput")
    rowmin_d = nc.dram_tensor("rowmin", [P, MT * NCH * 2], FP32, kind="ExternalOutput")
    # partition-min truncated at 32 rows on device; host finishes the min
    colmin_d = nc.dram_tensor("colmin", [32, B], FP32, kind="ExternalOutput")

    with tile.TileContext(nc) as tc:
        with (
            tc.tile_pool(name="consts", bufs=1) as consts,
            tc.tile_pool(name="yt8", bufs=2) as ytp,
            tc.tile_pool(name="thinp", bufs=2) as thinp,
            tc.tile_pool(name="work", bufs=3) as work,
            tc.tile_pool(name="pmm", bufs=8, space=bass.MemorySpace.PSUM) as pmm,
        ):
            rowmin_ch = consts.tile([P, MT * NCH * 2], FP32)
            thinX = consts.tile([4, RPC], BF16)
            nc.sync.dma_start(out=thinX[:], in_=thinx[:, :])
            XT = [
                consts.tile([P, RPC], BF16, tag=f"xt{d}", name=f"xt{d}")
                for d in range(DT)
            ]

            # ---- Phase A: load X^T (already bf16), interleaved with the
            # first Y chunk's loads below so both streams share the queues
            for dt in range(DT):
                for hf in range(2):
                    nc.sync.dma_start(
                        out=XT[dt][:, ts(hf, RPC // 2)],
                        in_=xt[ts(dt, P), ts(hf, RPC // 2)],
                    )

            # ---- Phase B: stream column chunks of Y^T ----
            for ch in range(NCH):
                j0 = ch * CHUNK
                thinY = thinp.tile([4, CHUNK], BF16, tag="thiny")
                nc.sync.dma_start(out=thinY[:], in_=thiny[:, j0 : j0 + CHUNK])
                yts = []
                for dt in range(DT):
                    ytile = ytp.tile([P, CHUNK], BF16, tag=f"yt{dt}", name=f"yt{dt}")
                    for hf in range(2):
                        nc.sync.dma_start(
                            out=ytile[:, ts(hf, HALF)],
                            in_=yt[ts(dt, P), j0 + hf * HALF : j0 + (hf + 1) * HALF],
                        )
                    yts.append(ytile)

                colmin = work.tile([P, CHUNK], FP32, tag="colmin")
                for m in range(MT):
                    for h in range(2):
                        hs = slice(h * HALF, (h + 1) * HALF)
                        ps = pmm.tile([P, HALF], FP32, tag="ps")
                        for dt in range(DT):
                            nc.tensor.matmul(
                                ps[:],
                                XT[dt][:, ts(m, P)],
                                yts[dt][:, hs],
                                start=(dt == 0),
                                stop=False,
                            )
                        nc.tensor.matmul(
                            ps[:], thinX[:, ts(m, P)], thinY[:, hs],
                            start=False, stop=True,
                        )
                        k = (m * NCH + ch) * 2 + h
                        nc.vector.tensor_reduce(
                            out=rowmin_ch[:, k : k + 1], in_=ps[:],
                            axis=AX.X, op=AL.min,
                        )
                        if m == 0:
                            nc.vector.tensor_copy(colmin[:, hs], ps[:])
                        else:
                            nc.vector.scalar_tensor_tensor(
                                colmin[:, hs], ps[:], 0.0, colmin[:, hs],
                                op0=AL.bypass, op1=AL.min,
                            )

                # partial min across partitions (128 -> 32); host finishes
                for s in (64, 32):
                    tmp = work.tile([64, CHUNK], FP32, tag="tree")
                    nc.sync.dma_start(out=tmp[:s, :], in_=colmin[s : 2 * s, :])
                    nc.vector.tensor_tensor(
                        colmin[0:s, :], colmin[0:s, :], tmp[:s, :], AL.min
                    )
                nc.sync.dma_start(
                    out=colmin_d[:, j0 : j0 + CHUNK], in_=colmin[0:32, :]
                )

            nc.sync.dma_start(out=rowmin_d[:, :], in_=rowmin_ch[:, :])
    if legalize:
        _legalize_waits(nc)
    return nc


_NC_CACHE = None


def _get_nc():
    global _NC_CACHE
    if _NC_CACHE is None:
        _NC_CACHE = build_bass()
    return _NC_CACHE


def _hi_lo(v):
    hi = v.astype(ml_dtypes.bfloat16)
    lo = (v - hi.astype(np.float64)).astype(ml_dtypes.bfloat16)
    return hi, lo


def _prep_inputs(X, Y):
    """Host-side sharding/layout: contraction-major bf16 operands + packed
    sq rows."""
    yt = np.ascontiguousarray(Y.T.astype(ml_dtypes.bfloat16))
    sqy = (Y.astype(np.float64) ** 2).sum(axis=1)
    sqy_hi, sqy_lo = _hi_lo(sqy)
    ones_b = np.ones(B, dtype=ml_dtypes.bfloat16)
    thiny = np.ascontiguousarray(np.stack([ones_b, ones_b, sqy_hi, sqy_lo]))

    in_maps = []
    for c in range(NCORES):
        Xs = X[c * RPC : (c + 1) * RPC]
        xt = np.ascontiguousarray((-2.0 * Xs).T.astype(ml_dtypes.bfloat16))
        sqx = (Xs.astype(np.float64) ** 2).sum(axis=1)
        sqx_hi, sqx_lo = _hi_lo(sqx)
        ones_r = np.ones(RPC, dtype=ml_dtypes.bfloat16)
        thinx = np.ascontiguousarray(np.stack([sqx_hi, sqx_lo, ones_r, ones_r]))
        in_maps.append({"xt": xt, "yt": yt, "thinx": thinx, "thiny": thiny})
    return in_maps


def kernel(input, target):
    X = np.ascontiguousarray(np.asarray(input, dtype=np.float32))
    Y = np.ascontiguousarray(np.asarray(target, dtype=np.float32))
    assert X.shape == (B, D) and Y.shape == (B, D)

    nc = _get_nc()
    in_maps = _prep_inputs(X, Y)
    try:
        res = run_bass_kernel_spmd(nc, in_maps, core_ids=list(range(NCORES))).results
    except Exception:
        # a prior process can leave a core wedged; one retry clears it
        res = run_bass_kernel_spmd(nc, in_maps, core_ids=list(range(NCORES))).results

    row_sum = np.float64(0.0)
    col_parts = []
    for r in res:
        rm = r["rowmin"].reshape(P, MT, NCH * 2).min(axis=2)
        row_sum += rm.astype(np.float64).sum()
        col_parts.append(r["colmin"].min(axis=0))
    col_min = np.min(np.stack(col_parts), axis=0).astype(np.float64)
    loss = (row_sum + col_min.sum()) / D / (2 * B)
    return np.asarray(loss, dtype=np.float32)
